# revision 18
# baseline (speedup 1.0000x reference)
"""AWQ W4A8 linear (x:[8,32,8192] f32, qweight:[8192,8192] int4-range int32,
w_scales/bias:[8192] f32) -> [8,32,8192] f32 on 8 trn2 NeuronCores.

Column-parallel sharding: qweight / w_scales / bias are split along N
(output channels) across the 8 cores; x — quantized per-token on the host
exactly as the reference does — and the per-token act_scales are
replicated. Each core computes an exact integer GEMM of
x_q [256,8192] @ qw_shard [8192,1024], applies the per-token/per-channel
dequant + bias epilogue, and writes its [256,1024] slice; the host
concatenates the slices.

Numerics: x_q in [-127,127] ships as bf16 and qw in [-8,7] ships as fp8e4
(both exactly representable), and the PE's mixed bf16 x fp8 matmul
accumulates exactly in fp32 PSUM (every product/sum is an integer < 2^24).
The epilogue result is stored as bf16 (rel err ~1e-3, well under the 2e-2
gate), halving output traffic.

The device program is raw Bass (no TileContext) with hand-placed
semaphores. Schedule (from trace analysis of the v1 kernel):
- Pre-barrier, the first weight group rides the SP DGE queue and the
  first activation piece rides the ACT queue, so BOTH rings spin up
  during the ~6.7us framework preamble.
- Junk warm-up matmuls on uninitialized SBUF keep the PE busy during the
  first-data DMA wait so the HAM clock is at full rate when real data
  lands (v1 paid ~3us of p-state ramp inside the stream).
- Constants (asc/ws/bs broadcasts, ~1MB) ride the DVE engine's queue:
  in v1 they sat between xq pieces on the ACT queue and stalled the PE
  2.5us waiting for piece 4.
- Weights stream through 6 SBUF slots with ramped DMA group sizes; the
  last group runs PSUM-tile-by-tile so dequant epilogues and bf16 output
  stores overlap the tail matmuls; the final tile's epilogue is split
  into two 256-wide halves whose stores leave on both queues.
"""

from contextlib import ExitStack

import numpy as np

import concourse.bass as bass
import concourse.mybir as mybir
import concourse.bass_utils as bass_utils
from concourse.dt import dt as cdt

N_CORES = 8
P = 128
B, S, K, N = 8, 32, 8192, 8192
TOK = B * S                      # 256 tokens
NL = N // N_CORES                # 1024 output channels per core
KC = K // P                      # 64 contraction chunks of 128
EPS = 1e-8

W_GROUPS = [2, 4, 6, 4] + [8] * 6     # weight k-chunks per DMA group
X_GROUPS = [4, 4] + [8] * 7           # activation k-chunks per DMA piece
NSLOT = 6                             # weight SBUF slots (capacity 8 chunks)
N_WARM = 45                           # junk matmuls to pre-warm the PE clock

assert sum(W_GROUPS) == KC and sum(X_GROUPS) == KC
assert max(W_GROUPS) <= 8

_cached = None


def _piece_of(c):
    acc = 0
    for i, gc in enumerate(X_GROUPS):
        if c < acc + gc:
            return i
        acc += gc
    raise ValueError(c)


def _build_nc():
    nc = bass.Bass(
        "TRN2",
        target_bir_lowering=False,
        debug=False,
        enable_asserts=False,
        num_devices=N_CORES,
    )
    dt = mybir.dt

    xq_d = nc.dram_tensor("xq", [P, KC, TOK], dt.bfloat16, kind="ExternalInput")
    qw_d = nc.dram_tensor("qw", [P, KC, NL], dt.float8e4, kind="ExternalInput")
    ws_d = nc.dram_tensor("ws", [P, NL], dt.float32, kind="ExternalInput")
    bs_d = nc.dram_tensor("bs", [P, NL], dt.float32, kind="ExternalInput")
    as_d = nc.dram_tensor("asc", [P, 2], dt.float32, kind="ExternalInput")
    out_d = nc.dram_tensor("out", [2, P, NL], dt.bfloat16, kind="ExternalOutput")

    ctx = ExitStack()
    xq_s = ctx.enter_context(nc.sbuf_tensor("xq_s", [P, KC, TOK], dt.bfloat16))
    w_s = ctx.enter_context(nc.sbuf_tensor("w_s", [P, NSLOT, 8, NL], dt.float8e4))
    ws_s = ctx.enter_context(nc.sbuf_tensor("ws_s", [P, NL], dt.float32))
    bs_s = ctx.enter_context(nc.sbuf_tensor("bs_s", [P, NL], dt.float32))
    as_s = ctx.enter_context(nc.sbuf_tensor("as_s", [P, 2], dt.float32))
    t_s = ctx.enter_context(nc.sbuf_tensor("t_s", [P, 4, 512], dt.float32))
    o_s = ctx.enter_context(nc.sbuf_tensor("o_s", [P, 4, 512], dt.bfloat16))

    ps = [
        ctx.enter_context(nc.psum_tensor(f"ps{i}", [P, 512], dt.float32))
        for i in range(4)  # (m,n): 00,01,10,11
    ]
    ps_warm = ctx.enter_context(nc.psum_tensor("ps_warm", [P, 512], dt.float32))

    sems = {}

    def sem(name):
        sems[name] = ctx.enter_context(nc.semaphore(name))
        return sems[name]

    s_wg = [sem(f"s_wg{g}") for g in range(len(W_GROUPS))]
    s_xq = [sem(f"s_xq{i}") for i in range(len(X_GROUPS))]
    s_cst = sem("s_cst")
    s_pe = sem("s_pe")
    s_ps = [sem(f"s_ps{i}") for i in range(4)]
    s_ep = [sem(f"s_ep{i}") for i in range(4)]
    s_out = sem("s_out")
    s_dve = sem("s_dve")

    w_starts = np.cumsum([0] + W_GROUPS).tolist()
    x_starts = np.cumsum([0] + X_GROUPS).tolist()
    TILES = [(0, 0), (0, 1), (1, 0), (1, 1)]

    # Zero our semaphores (a previous execution of this NEFF leaves them at
    # their final values), then barrier so no engine runs ahead. The first
    # data DMAs are issued INSIDE the block right after the barrier: the
    # descriptor writes land within ~1us of a pre-barrier issue, but the
    # barrier itself clears ~2us earlier (pre-barrier issues gate it), so
    # the warm-up matmuls start sooner and the PE clock is promoted before
    # real data arrives. It also removes the clear-vs-completion race.
    nums = sorted(s.num for s in sems.values())
    lo = 0
    while lo < len(nums):
        hi = lo
        while hi + 1 < len(nums) and nums[hi + 1] == nums[hi] + 1:
            hi += 1
        nc.gpsimd.sem_clear(range(nums[lo], nums[hi] + 1))
        lo = hi + 1
    nc.all_engine_barrier()

    with nc.Block() as block:

        @block.sync
        def _(sync):
            sync.dma_start(
                w_s[:, 0, : W_GROUPS[0], :], qw_d.ap()[:, 0 : W_GROUPS[0], :]
            ).then_inc(s_wg[0], 16)
            sync.dma_start(
                xq_s[:, 0 : X_GROUPS[0], :], xq_d.ap()[:, 0 : X_GROUPS[0], :]
            ).then_inc(s_xq[0], 16)
            for g, gc in enumerate(W_GROUPS[1:], start=1):
                if g >= NSLOT:
                    sync.wait_ge(s_pe, g - NSLOT + 1)
                c0 = w_starts[g]
                sync.dma_start(
                    w_s[:, g % NSLOT, :gc, :], qw_d.ap()[:, c0 : c0 + gc, :]
                ).then_inc(s_wg[g], 16)
            # stores for tiles 0 and 2 (tile 2 is the final tail store)
            sync.wait_ge(s_ep[0], 1)
            sync.dma_start(out_d.ap()[0][:, 0:512], o_s[:, 0, :]).then_inc(
                s_out, 16
            )
            sync.wait_ge(s_ep[2], 1)
            sync.dma_start(out_d.ap()[1][:, 0:512], o_s[:, 2, :]).then_inc(
                s_out, 16
            )

        @block.scalar
        def _(scalar):
            def xq_dma(i):
                xo, xc = x_starts[i], X_GROUPS[i]
                scalar.dma_start(
                    xq_s[:, xo : xo + xc, :], xq_d.ap()[:, xo : xo + xc, :]
                ).then_inc(s_xq[i], 16)

            for i in range(1, len(X_GROUPS)):
                xq_dma(i)
            # Constants AFTER all xq pieces (in v1 they sat between pieces
            # 3 and 4 and stalled the PE 2.5us; they are only needed by the
            # epilogues at ~60us).
            scalar.dma_start(as_s[:], as_d.ap()).then_inc(s_cst, 16)
            scalar.dma_start(ws_s[:], ws_d.ap()).then_inc(s_cst, 16)
            scalar.dma_start(bs_s[:], bs_d.ap()).then_inc(s_cst, 16)
            # stores for tiles 3 (finishes first) and 1
            scalar.wait_ge(s_ep[3], 1)
            scalar.dma_start(
                out_d.ap()[1][:, 512:1024], o_s[:, 3, :]
            ).then_inc(s_out, 16)
            scalar.wait_ge(s_ep[1], 1)
            scalar.dma_start(
                out_d.ap()[0][:, 512:1024], o_s[:, 1, :]
            ).then_inc(s_out, 16)

        @block.tensor
        def _(tensor):
            # Warm-up: junk matmuls on uninitialized SBUF into a scratch
            # PSUM bank while the first data DMAs are in flight. Keeps the
            # PE busy from the moment the barrier clears so the HAM clock
            # is at full rate when the real stream starts.
            for _ in range(N_WARM):
                tensor.matmul(
                    ps_warm.ap()[:, 0:64],
                    xq_s[:, KC - 1, 0:P],
                    w_s[:, NSLOT - 1, 7, 0:64],
                    start=True,
                    stop=True,
                )

            cur_piece = -1

            def mm(c, m, n, idx=None, inc_pe=False, stop=None):
                nonlocal cur_piece
                pc = _piece_of(c)
                if pc != cur_piece:
                    tensor.wait_ge(s_xq[pc], 16)
                    cur_piece = pc
                g = next(i for i in range(len(W_GROUPS)) if w_starts[i + 1] > c)
                inst = tensor.matmul(
                    ps[2 * m + n].ap(),
                    xq_s[:, c, P * m : P * (m + 1)],
                    w_s[:, g % NSLOT, c - w_starts[g], 512 * n : 512 * (n + 1)],
                    start=(c == 0),
                    stop=(c == KC - 1),
                )
                if idx is not None:
                    inst.then_inc(s_ps[idx], 1)
                if inc_pe:
                    inst.then_inc(s_pe, 1)

            for g, gc in enumerate(W_GROUPS[:-1]):
                tensor.wait_ge(s_wg[g], 16)
                c0 = w_starts[g]
                for j in range(gc):
                    for m in range(2):
                        for n in range(2):
                            mm(
                                c0 + j,
                                m,
                                n,
                                inc_pe=(j == gc - 1 and m == 1 and n == 1),
                            )

            # last group: tile-by-tile so epilogues overlap the tail
            # matmuls; order 3,0,1,2 puts three epilogues under the tail
            # MMs and leaves only tile 2's epilogue + store as the serial
            # tail
            g = len(W_GROUPS) - 1
            gc = W_GROUPS[g]
            c0 = w_starts[g]
            tensor.wait_ge(s_wg[g], 16)
            for idx in (3, 0, 1, 2):
                m, n = TILES[idx]
                for j in range(gc):
                    mm(c0 + j, m, n, idx=(idx if j == gc - 1 else None))

        @block.vector
        def _(vector):
            vector.wait_ge(s_cst, 48)
            ndve = 0

            def epilogue(idx, m, n, fsl, ep_i, ps_wait):
                # out = psum * asc[m] * ws + bs, written as bf16
                nonlocal ndve
                if ps_wait is not None:
                    vector.wait_ge(s_ps[ps_wait], 1)
                nsl = slice(512 * n + fsl.start, 512 * n + fsl.stop)
                vector.scalar_tensor_tensor(
                    t_s[:, idx, fsl],
                    ps[2 * m + n].ap()[:, fsl],
                    as_s[:, m : m + 1],
                    ws_s[:, nsl],
                    mybir.AluOpType.mult,
                    mybir.AluOpType.mult,
                ).then_inc(s_dve, 1)
                ndve += 1
                # DVE is deeply pipelined: same-engine RAW needs a sem
                vector.wait_ge(s_dve, ndve)
                vector.tensor_add(
                    o_s[:, idx, fsl], t_s[:, idx, fsl], bs_s[:, nsl]
                ).then_inc(s_ep[ep_i], 1)

            full = slice(0, 512)
            epilogue(3, 1, 1, full, 3, 3)
            epilogue(0, 0, 0, full, 0, 0)
            epilogue(1, 0, 1, full, 1, 1)
            epilogue(2, 1, 0, full, 2, 2)

    return nc, ctx


def _prep_inputs(x, qweight, w_scales, bias):
    bf16 = cdt.np(mybir.dt.bfloat16)
    fp8 = cdt.np(mybir.dt.float8e4)

    x2 = np.asarray(x, dtype=np.float32).reshape(TOK, K)
    max_abs = np.max(np.abs(x2), axis=-1, keepdims=True)
    act_scales = np.maximum(max_abs / np.float32(127.0), np.float32(EPS)).astype(
        np.float32
    )
    x_q = np.clip(np.round(x2 / act_scales), -127, 127).astype(np.float32)

    # [TOK, K] -> K-major [P, KC, TOK]: xq[p, c, t] = x_q[t, c*128 + p]
    xq = np.ascontiguousarray(
        x_q.T.reshape(KC, P, TOK).transpose(1, 0, 2).astype(bf16)
    )

    # act_scales arranged per m-tile: asc[p, m] = act_scales[m*128 + p]
    asc = np.ascontiguousarray(act_scales.reshape(2, P).T.astype(np.float32))

    # int4-range weights are exactly representable in fp8 e4m3
    qw8 = np.asarray(qweight, dtype=np.int8).astype(fp8)
    w_scales = np.asarray(w_scales, dtype=np.float32)
    bias = np.asarray(bias, dtype=np.float32)

    in_maps = []
    for i in range(N_CORES):
        sl = slice(i * NL, (i + 1) * NL)
        # [K, NL] -> p-major [P, KC, NL]: qw[p, c, n] = shard[c*128 + p, n]
        shard = qw8[:, sl].reshape(KC, P, NL).transpose(1, 0, 2)
        in_maps.append(
            {
                "xq": xq,
                "qw": np.ascontiguousarray(shard),
                "ws": np.ascontiguousarray(
                    np.broadcast_to(w_scales[sl][None, :], (P, NL))
                ),
                "bs": np.ascontiguousarray(
                    np.broadcast_to(bias[sl][None, :], (P, NL))
                ),
                "asc": asc,
            }
        )
    return in_maps


def kernel(x, qweight, w_scales, bias):
    global _cached
    if _cached is None:
        _cached = _build_nc()
    nc, _ = _cached

    in_maps = _prep_inputs(x, qweight, w_scales, bias)
    res = None
    err = None
    for _ in range(3):  # retry transient device errors
        try:
            res = bass_utils.run_bass_kernel_spmd(
                nc, in_maps, core_ids=list(range(N_CORES))
            )
            break
        except Exception as e:  # noqa: BLE001
            err = e
    if res is None:
        raise err

    out = np.empty((TOK, N), dtype=np.float32)
    for i in range(N_CORES):
        out[:, i * NL : (i + 1) * NL] = (
            res.results[i]["out"].astype(np.float32).reshape(TOK, NL)
        )
    return out.reshape(B, S, N)
